# revision 19
# baseline (speedup 1.0000x reference)
"""Fused sp2norm-MHA kernel for Trainium2, 8 NeuronCores.

Model (per reference):
    qkv = x @ W_qkv.T ; split heads (H=16, hs=64)
    s = (q @ k.T) / sqrt(hs);  w = softplus(s) causal-masked
    out_h = (w @ v) / ||w||_row ;  out = concat(out_h) @ W_proj.T + b_proj

Sharding: core c = (b, g) with b = c // 4 (batch), g = c % 4 (head group of 4).
Each core computes its batch's QKV for its 4 heads, the attention, and a
partial projection over its 256 feature channels. The host sums the 4 partial
projections per batch and adds the bias (the unshard step).

Schedule: the ACT engine (softplus = Exp then Ln over every causal score
element, ~143us at 1 elem/lane/cycle) is the roofline for this problem, so the
kernel is built around keeping ACT busy from ~13us on:
  - i-chunk-major pipeline: attention for chunk ic starts as soon as its q/k
    slabs exist; QKV for later chunks, the V tiles, and the output projection
    are emitted as "filler" PE work interleaved between attention blocks,
    paced adaptively and deferred toward the late (ACT-rich) chunks.
  - score/out/norm matmul pairs use disjoint PE row/col tiles (K=64 row-split,
    M=64 / M=1 col-split) so each pair runs concurrently (~1 matmul time).
  - lag-1 software pipeline: unit k+1's scores+softplus are emitted before
    unit k's out/norm matmuls so the PE stream never parks on Ln(k) with
    ACT's next input missing.
  - softplus Exp reads score pairs [128, 2N] straight from 2 PSUM banks; for
    full blocks two j-blocks share one Ln over [128, 4N] (fp16 intermediate)
    to amortize ACT instruction overhead.
  - norm^2 rows for the two head-pairs land in one PSUM bank at partitions
    {0,64} (pair 0) and {32,96} (pair 1); one Ln+Exp epilogue per i-chunk
    computes every rsqrt; out.T is staged unnormalized to SBUF so the po
    PSUM bank recycles immediately.
  - host pre-lays inputs out so every input DMA moves >=2KB contiguous per
    partition line (DMA granularity sweet spot).
PSUM budget: scores [128,2048] (4 banks) + po (1) + pn (1) + aux ring
(qkv/v/proj/bcast, 2) = 8 banks exactly.
"""

import collections
import numpy as np
import ml_dtypes

import concourse.bacc as bacc
import concourse.tile as tile
import concourse.mybir as mybir
from concourse.bass_utils import run_bass_kernel_spmd

# The act-table-set chooser assigns each activation the FIRST set containing
# its function; with the default ordering Exp -> exp_and_others and
# Ln -> natural_log, so alternating Exp/Ln thrashes ACT_TABLE_LOAD (~1.3us
# each, >100 loads). Reorder so the combined Exp+Ln set is preferred.
_orig_get_tables = bacc.get_activation_tables


def _tables_ln_exp_first(arch):
    t = _orig_get_tables(arch)
    key = "natural_log_exp_and_others"
    if key not in t:
        return t
    # Keep dict ORDER (set ids are positional); drop Exp/Ln from every other
    # set so the combined set is the unique candidate for both.
    exp = mybir.ActivationFunctionType.Exp
    ln = mybir.ActivationFunctionType.Ln
    out = {}
    for k, fns in t.items():
        out[k] = fns if k == key else (set(fns) - {exp, ln})
    return out


bacc.get_activation_tables = _tables_ln_exp_first

dt = mybir.dt
F32, F16, BF16 = dt.float32, dt.float16, dt.bfloat16
AF = mybir.ActivationFunctionType

B, T, C, H, HS = 2, 2048, 1024, 16, 64
NCORES = 8
SCALE = 1.0 / np.sqrt(HS)

_CACHE = {}


def _build():
    nc = bacc.Bacc(None, target_bir_lowering=False)

    # host-prearranged layouts: partition dim first, >=2KB contiguous lines
    xT = nc.dram_tensor("xT", [128, 4, 4, 1024], BF16, kind="ExternalInput")
    wqk = nc.dram_tensor("wqk", [128, 4, 1024], BF16, kind="ExternalInput")
    wv = nc.dram_tensor("wv", [128, 2, 1024], BF16, kind="ExternalInput")
    wp = nc.dram_tensor("wp", [128, 2, 1024], BF16, kind="ExternalInput")
    mtri = nc.dram_tensor("mtri", [128, 128], BF16, kind="ExternalInput")
    out = nc.dram_tensor("out", [T, C], F32, kind="ExternalOutput")

    with tile.TileContext(nc) as tc:
        with (
            tc.tile_pool(name="cst", bufs=1) as cst,
            tc.tile_pool(name="data", bufs=1) as data,
            tc.tile_pool(name="ps", bufs=1, space="PSUM") as ps_pool,
            tc.tile_pool(name="pso", bufs=1, space="PSUM") as pso_pool,
            tc.tile_pool(name="psn", bufs=1, space="PSUM") as psn_pool,
            tc.tile_pool(name="aux", bufs=2, space="PSUM") as aux_pool,
            tc.tile_pool(name="we", bufs=3) as we,
            tc.tile_pool(name="epi", bufs=2) as epi,
            tc.tile_pool(name="outp", bufs=6) as outp,
        ):
            # ---- act-table preload: a tiny Exp at t~0 pulls the single
            # ACT_TABLE_LOAD (natural_log_exp_and_others) off the critical path
            warm = cst.tile([1, 2], F32)
            nc.vector.memset(warm, 0.0)
            warm2 = cst.tile([1, 2], F32)
            nc.scalar.activation(warm2, warm, AF.Exp)

            # ---- constants / weights (DMA issue order == need order) ----
            mtri_sb = cst.tile([128, 128], BF16)
            nc.sync.dma_start(mtri_sb, mtri[:])
            ones_n = cst.tile([128, 1], BF16)
            nc.vector.memset(ones_n, 1.0)
            ones_b = cst.tile([128, 64], BF16)
            nc.vector.memset(ones_b, 1.0)

            # wqk_sb[:, cp, c2*512 + jb*128 + j] = W_qkv row (2cp+c2)*128+pi
            wqk_sb = cst.tile([128, 4, 1024], BF16)
            # xT_sb[:, tq, cp, c2*512 + tj] = x.T[(2cp+c2)*128+pi, tq*512+tj]
            xT_sb = data.tile([128, 4, 4, 1024], BF16)
            wv_sb = cst.tile([128, 2, 1024], BF16)
            wp_sb = cst.tile([128, 2, 1024], BF16)
            # first-needed tensors in halves (partial compute start), the
            # rest as single >=0.5MB transfers (descriptor-amortized)
            for h in range(2):
                nc.sync.dma_start(wqk_sb[:, 2 * h:2 * h + 2, :],
                                  wqk[:, 2 * h:2 * h + 2, :])
                nc.sync.dma_start(xT_sb[:, 0, 2 * h:2 * h + 2, :],
                                  xT[:, 0, 2 * h:2 * h + 2, :])
            nc.sync.dma_start(wv_sb, wv[:])
            for tq in range(1, 4):
                nc.sync.dma_start(xT_sb[:, tq, :, :], xT[:, tq, :, :])
            nc.sync.dma_start(wp_sb, wp[:])

            def x_mv(cb, tcc):     # moving [128, 512] for q/k slab
                return xT_sb[:, tcc, cb // 2,
                             (cb % 2) * 512:(cb % 2) * 512 + 512]

            def x_st(cb, tb):      # stationary [128, 128] for v tile
                off = (cb % 2) * 512 + (tb % 4) * 128
                return xT_sb[:, tb // 4, cb // 2, off:off + 128]

            # qkT: block 0,1 = q head-pairs; block 2,3 = k head-pairs.
            # Partition rows (h%2)*64..+64 inside each block = one head.
            qkT = data.tile([128, 4, 2048], BF16)
            v_sb = data.tile([128, 16, 256], BF16)
            yTu = data.tile([128, 2, 2048], BF16)   # unnormalized out.T
            yT = data.tile([128, 2, 2048], BF16)    # normalized out.T

            # scores psum: one 4-bank tile, manually split in two
            # double-buffered halves (range-level deps track the halves).
            ps_all = ps_pool.tile([128, 2048], F32)

            # ---------- PE filler units (QKV slabs / V tiles / proj) -------
            def qk_slab(jb, tcc, t0=0, t1=512):
                pq = aux_pool.tile([128, 512], F32, tag="aux")
                for cb in range(8):
                    nc.tensor.matmul(
                        pq[:, t0:t1],
                        wqk_sb[:, cb // 2,
                               (cb % 2) * 512 + jb * 128:
                               (cb % 2) * 512 + jb * 128 + 128],
                        x_mv(cb, tcc)[:, t0:t1],
                        start=(cb == 0), stop=(cb == 7),
                    )
                nc.vector.tensor_copy(
                    qkT[:, jb, tcc * 512 + t0:tcc * 512 + t1],
                    pq[:, t0:t1])

            def v_tile(tb):
                pv = aux_pool.tile([128, 512], F32, tag="aux")
                for cb in range(8):
                    nc.tensor.matmul(
                        pv[:, 0:256],
                        x_st(cb, tb),
                        wv_sb[:, cb // 4,
                              (cb % 4) * 256:(cb % 4) * 256 + 256],
                        start=(cb == 0), stop=(cb == 7),
                    )
                nc.vector.tensor_copy(v_sb[:, tb, :], pv[:, 0:256])

            def proj_half(tcc, nk, pp=None, scalar_copy=False):
                if pp is None:
                    pp = aux_pool.tile([128, 512], F32, tag="aux")
                for kb in range(2):
                    nc.tensor.matmul(
                        pp,
                        yT[:, kb, tcc * 128:(tcc + 1) * 128],
                        wp_sb[:, kb, nk * 512:(nk + 1) * 512],
                        start=(kb == 0), stop=(kb == 1),
                    )
                os_ = outp.tile([128, 512], F32, tag="os")
                if scalar_copy:
                    nc.scalar.copy(os_, pp)
                else:
                    nc.vector.tensor_copy(os_, pp)
                nc.sync.dma_start(
                    out[tcc * 128:(tcc + 1) * 128, nk * 512:(nk + 1) * 512],
                    os_)

            fillers = collections.deque()   # (key, fn)
            emitted = set()

            def pop_fillers(n):
                for _ in range(n):
                    if not fillers:
                        return
                    key, fn = fillers.popleft()
                    emitted.add(key)
                    fn()

            def ensure(*keys):
                """Emit queued fillers (in order) until every key has run.
                Correctness barrier: consumers call this right before the
                first instruction that reads a filler's output."""
                while any(k not in emitted for k in keys) and fillers:
                    pop_fillers(1)

            # ---------------- attention building blocks -------------------
            def scores_block(hp, ic, jb, half):
                """Score pair for j-block jb against i-chunk ic; returns
                (ps half AP, N)."""
                qblk, kblk = hp, 2 + hp
                m = jb - 4 * ic
                N = 512 if m < 0 else 512 - 128 * m
                ioff = ic * 512 + (512 - N)
                ps_ = ps_all[:, half * 1024:half * 1024 + 1024]
                nc.tensor.matmul(
                    ps_[:, 0:N],
                    qkT[0:64, kblk, jb * 128:(jb + 1) * 128],
                    qkT[0:64, qblk, ioff:ioff + N],
                    start=True, stop=True,
                )
                nc.tensor.matmul(
                    ps_[:, 512:512 + N],
                    qkT[64:128, kblk, jb * 128:(jb + 1) * 128],
                    qkT[64:128, qblk, ioff:ioff + N],
                    start=True, stop=True,
                )
                return ps_, N

            def softplus_single(ps_, N, diag):
                """softplus = Ln(Exp(s/8) + 1), fp16 intermediate; one block.
                A-half at w[:, 0:N], B-half at w[:, N:2N]."""
                e = we.tile([128, 1024], F16, tag="e1")
                w = we.tile([128, 1024], BF16, tag="w1")
                w2 = we.tile([128, 1024], BF16, tag="ww1")
                if N == 512:
                    nc.scalar.activation(e, ps_, AF.Exp, scale=SCALE)
                else:
                    ps3 = ps_.rearrange("p (b n) -> p b n", b=2)[:, :, 0:N]
                    e3 = e[:, 0:2 * N].rearrange("p (b n) -> p b n", b=2)
                    nc.scalar.activation(e3, ps3, AF.Exp, scale=SCALE)
                nc.scalar.activation(w[:, 0:2 * N], e[:, 0:2 * N],
                                     AF.Ln, bias=1.0)
                if diag:
                    nc.vector.tensor_mul(w[:, 0:128], w[:, 0:128], mtri_sb)
                    nc.vector.tensor_mul(w[:, N:N + 128], w[:, N:N + 128],
                                         mtri_sb)
                nc.vector.tensor_mul(w2[:, 0:2 * N], w[:, 0:2 * N],
                                     w[:, 0:2 * N])
                return w, w2

            def out_norm_block(hp, ic, njb, jb, w, w2, woff, po, pn):
                """out.T and norm^2 accumulation for one block; w/w2 tile
                slices start at free offset woff, layout [A(N) | B(N)]."""
                m = jb - 4 * ic
                N = 512 if m < 0 else 512 - 128 * m
                start, stop = jb == 0, jb == njb - 1
                hA, hB = 2 * hp, 2 * hp + 1
                nc.tensor.matmul(
                    po[0:64, 512 - N:512],
                    v_sb[:, jb, hA * 64:hA * 64 + 64],
                    w[:, woff:woff + N],
                    start=start, stop=stop, tile_position=(0, 0),
                )
                nc.tensor.matmul(
                    po[64:128, 512 - N:512],
                    v_sb[:, jb, hB * 64:hB * 64 + 64],
                    w[:, woff + N:woff + 2 * N],
                    start=start, stop=stop, tile_position=(0, 64),
                )
                rA, rB = 32 * hp, 64 + 32 * hp
                nc.tensor.matmul(
                    pn[rA:rA + 1, 512 - N:512], ones_n,
                    w2[:, woff:woff + N],
                    start=start, stop=stop, tile_position=(0, rA),
                )
                nc.tensor.matmul(
                    pn[rB:rB + 1, 512 - N:512], ones_n,
                    w2[:, woff + N:woff + 2 * N],
                    start=start, stop=stop, tile_position=(0, rB),
                )

            # ------------------------- pipeline ---------------------------
            # prologue: only what the very first score block needs — full q
            # slab for pair 0 plus the first 128 k columns
            qk_slab(0, 0)           # q pair 0
            qk_slab(2, 0, 0, 128)   # k pair 0, j-block 0 only
            emitted.update({"s0_0", "s2_0"})
            fillers.append(("k0rest", lambda: qk_slab(2, 0, 128, 512)))
            for tb_ in range(4):
                fillers.append(("v%d" % tb_, lambda tb_=tb_: v_tile(tb_)))
            fillers.append(("s1_0", lambda: qk_slab(1, 0)))   # q pair 1
            fillers.append(("s3_0", lambda: qk_slab(3, 0)))   # k pair 1

            flip = 0
            pending_epi = [None]
            pending = None

            # lag-1 software pipeline (crosses chunk boundaries): emit unit
            # k+1's scores+softplus BEFORE unit k's out/norm matmuls, so the
            # PE stream never parks on Ln(k) with ACT's next input missing.
            def emit_out_norm(pend):
                hp_, jbs_, w_, w2_, po_, ic_, njb_, pn_ = pend
                ensure(*("v%d" % jb_ for jb_ in jbs_))
                for i_, jb_ in enumerate(jbs_):
                    out_norm_block(hp_, ic_, njb_, jb_, w_, w2_, 1024 * i_,
                                   po_, pn_)
                if jbs_[-1] == njb_ - 1:
                    # stage unnormalized out.T; frees po for the next pair
                    nc.vector.tensor_copy(
                        yTu[:, hp_, ic_ * 512:(ic_ + 1) * 512], po_)

            for ic in range(4):
                # queue this chunk's filler PE work; projection is deferred
                # toward the late, ACT-rich chunks.
                if ic > 0:
                    # this chunk's v tiles and pair-1 q/k slabs (due mid-chunk)
                    for tb_ in range(4 * ic, 4 * ic + 4):
                        fillers.append(
                            ("v%d" % tb_, lambda tb_=tb_: v_tile(tb_)))
                    fillers.append(
                        ("s1_%d" % ic, lambda ic=ic: qk_slab(1, ic)))
                    fillers.append(
                        ("s3_%d" % ic, lambda ic=ic: qk_slab(3, ic)))
                if ic < 3:
                    # next chunk's pair-0 q/k slabs (due at its start)
                    fillers.append(
                        ("s0_%d" % (ic + 1),
                         lambda tn=ic + 1: qk_slab(0, tn)))
                    fillers.append(
                        ("s2_%d" % (ic + 1),
                         lambda tn=ic + 1: qk_slab(2, tn)))
                for pic in ((0,) if ic == 2 else (1, 2) if ic == 3 else ()):
                    for tcc_ in range(4 * pic, 4 * pic + 4):
                        for nk_ in range(2):
                            fillers.append(
                                ("p%d_%d" % (tcc_, nk_),
                                 lambda tcc_=tcc_, nk_=nk_:
                                 proj_half(tcc_, nk_)))

                njb = 4 * ic + 4
                units_left = 2 * (2 * ic + 4)
                pn = psn_pool.tile([128, 512], F32, tag="pn")

                first_unit = True
                for hp in range(2):
                    po = pso_pool.tile([128, 512], F32, tag="po")
                    jb = 0
                    while jb < njb:
                        jb_max = jb + 1 if jb + 1 < 4 * ic else jb
                        ensure("s%d_%d" % (hp, ic),
                               "s%d_%d" % (2 + hp, jb_max // 4))
                        if hp == 0 and jb > 0:
                            ensure("k0rest")
                        if jb + 1 < 4 * ic:  # merge two full blocks
                            psA, _ = scores_block(hp, ic, jb, 0)
                            psB, _ = scores_block(hp, ic, jb + 1, 1)
                            # per-half Exp (finer WAR release of the score
                            # banks), one Ln over both halves
                            e = we.tile([128, 2048], F16, tag="e2")
                            w = we.tile([128, 2048], BF16, tag="w2")
                            w2 = we.tile([128, 2048], BF16, tag="ww2")
                            nc.scalar.activation(e[:, 0:1024], psA, AF.Exp,
                                                 scale=SCALE)
                            nc.scalar.activation(e[:, 1024:2048], psB, AF.Exp,
                                                 scale=SCALE)
                            nc.scalar.activation(w, e, AF.Ln, bias=1.0)
                            nc.vector.tensor_mul(w2, w, w)
                            cur = (hp, (jb, jb + 1), w, w2, po, ic, njb, pn)
                            jb += 2
                        else:
                            m = jb - 4 * ic
                            ps_, N = scores_block(hp, ic, jb, flip)
                            flip ^= 1
                            w, w2 = softplus_single(ps_, N, m >= 0)
                            cur = (hp, (jb,), w, w2, po, ic, njb, pn)
                            jb += 1
                        # adaptive pacing: leave no backlog for chunk end
                        quota = max(1, -(-len(fillers) // max(units_left, 1)))
                        pop_fillers(min(quota, 2))
                        if pending is not None:
                            emit_out_norm(pending)
                            pop_fillers(1 if quota > 2 else 0)
                        pending = cur
                        if first_unit and pending_epi[0] is not None:
                            # previous chunk's epilogue — emitted after this
                            # chunk's first Exp is queued on ACT, and after
                            # the previous chunk's final out/norm (pending)
                            pending_epi[0]()
                            pending_epi[0] = None
                        first_unit = False
                        units_left -= 1

                # ---- chunk epilogue: rsqrt(norm2) for all 4 heads ----
                # rsqrt = Exp(-0.5 * Ln(x)): stays in the Exp/Ln set.
                # Rows {0,64} = pair 0, {32,96} = pair 1; unread rows compute
                # garbage harmlessly (ACT cost is free-dim bound). Deferred
                # until the next chunk's first Exp is queued so the pb
                # matmuls (which park on ACT's rsqrt) never delay it.
                def chunk_epilogue(ic=ic, pn=pn):
                    nrm = epi.tile([128, 512], F32, tag="nrm")
                    nc.scalar.activation(nrm, pn, AF.Ln)
                    rs = epi.tile([128, 512], BF16, tag="rs")
                    nc.scalar.activation(rs, nrm, AF.Exp, scale=-0.5)
                    for hp in range(2):
                        rA, rB = 32 * hp, 64 + 32 * hp
                        pb = aux_pool.tile([128, 512], F32, tag="aux")
                        nc.tensor.matmul(pb[0:64, :], ones_b[rA:rA + 1, :],
                                         rs[rA:rA + 1, :],
                                         start=True, stop=True,
                                         tile_position=(rA, 0))
                        nc.tensor.matmul(pb[64:128, :], ones_b[rB:rB + 1, :],
                                         rs[rB:rB + 1, :],
                                         start=True, stop=True,
                                         tile_position=(rB, 64))
                        rb = epi.tile([128, 512], F32, tag="rb")
                        nc.vector.tensor_copy(rb, pb)
                        nc.vector.tensor_mul(
                            yT[:, hp, ic * 512:(ic + 1) * 512],
                            yTu[:, hp, ic * 512:(ic + 1) * 512], rb)

                if ic < 3:
                    pending_epi[0] = chunk_epilogue
                else:
                    emit_out_norm(pending)
                    pending = None
                    chunk_epilogue()
                    pop_fillers(len(fillers))

            # last chunk's projection: rotate the accumulator through the
            # aux ring plus the now-idle po/pn banks (4-deep pipeline with
            # proper per-tile release, unlike slicing one big psum tile).
            tail_pp = [
                lambda: aux_pool.tile([128, 512], F32, tag="aux", name="tpa"),
                lambda: aux_pool.tile([128, 512], F32, tag="aux", name="tpb"),
                lambda: pso_pool.tile([128, 512], F32, tag="po", name="tpo"),
                lambda: psn_pool.tile([128, 512], F32, tag="pn", name="tpn"),
            ]
            k = 0
            for tcc_ in range(12, 16):
                for nk_ in range(2):
                    proj_half(tcc_, nk_, pp=tail_pp[k % 4]())
                    k += 1

    nc.compile()
    return nc


def _prep_inputs(x, W_qkv, W_proj):
    """Host-side shard + layout prep. Returns per-core input maps.

    Layouts are DMA-friendly: partition index first, then chunk indices so
    every dma_start moves a contiguous >=2KB line per partition.
    """
    bf = ml_dtypes.bfloat16
    mtri = np.triu(np.ones((128, 128), dtype=np.float32)).astype(bf)
    in_maps = []
    for core in range(NCORES):
        b, g = core // 4, core % 4
        heads = range(4 * g, 4 * g + 4)
        # W_qkv rows: q = h*64.., k = C + h*64.., v = 2C + h*64..
        q_rows = np.concatenate([np.arange(h * HS, (h + 1) * HS) for h in heads])
        wqk = np.concatenate(
            [W_qkv[q_rows, :].T, W_qkv[C + q_rows, :].T], axis=1)  # [C, 512]
        wv = W_qkv[2 * C + q_rows, :].T                            # [C, 256]
        wp = W_proj[:, q_rows].T                                   # [256, C]

        xT_ = np.ascontiguousarray(x[b].T).astype(bf)              # [C, T]
        # [128, tq, cp, c2*512+tj]
        xh = xT_.reshape(4, 2, 128, 4, 512).transpose(2, 3, 0, 1, 4)
        xh = np.ascontiguousarray(xh).reshape(128, 4, 4, 1024)
        # [128, cp, c2*512+j]
        wqkh = wqk.astype(bf).reshape(4, 2, 128, 512).transpose(2, 0, 1, 3)
        wqkh = np.ascontiguousarray(wqkh).reshape(128, 4, 1024)
        # [128, cq, c4*256+j]
        wvh = wv.astype(bf).reshape(2, 4, 128, 256).transpose(2, 0, 1, 3)
        wvh = np.ascontiguousarray(wvh).reshape(128, 2, 1024)
        # [128, po, e]
        wph = np.ascontiguousarray(
            wp.astype(bf).reshape(2, 128, 1024).transpose(1, 0, 2))

        in_maps.append({
            "xT": xh, "wqk": wqkh, "wv": wvh, "wp": wph, "mtri": mtri,
        })
    return in_maps


def _run(in_maps, trace=False, trace_cores=None):
    if "nc" not in _CACHE:
        _CACHE["nc"] = _build()
    return run_bass_kernel_spmd(
        _CACHE["nc"], in_maps, core_ids=list(range(NCORES)),
        trace=trace, trace_cores=trace_cores,
    )


def kernel(x, W_qkv, W_proj, b_proj):
    x = np.asarray(x, dtype=np.float32)
    W_qkv = np.asarray(W_qkv, dtype=np.float32)
    W_proj = np.asarray(W_proj, dtype=np.float32)
    b_proj = np.asarray(b_proj, dtype=np.float32)

    res = _run(_prep_inputs(x, W_qkv, W_proj)).results
    out = np.zeros((B, T, C), dtype=np.float64)
    for core in range(NCORES):
        out[core // 4] += np.asarray(res[core]["out"], dtype=np.float64)
    out += b_proj.astype(np.float64)
    return out.astype(np.float32)


# revision 21
# speedup vs baseline: 1.1743x; 1.1743x over previous
"""Fused sp2norm-MHA kernel for Trainium2, 8 NeuronCores.

Model (per reference):
    qkv = x @ W_qkv.T ; split heads (H=16, hs=64)
    s = (q @ k.T) / sqrt(hs);  w = softplus(s) causal-masked
    out_h = (w @ v) / ||w||_row ;  out = concat(out_h) @ W_proj.T + b_proj

Sharding: core c = (b, g) with b = c // 4 (batch), g = c % 4 (head group of 4).
Each core computes its batch's QKV for its 4 heads, the attention, and a
partial projection over its 256 feature channels. The host sums the 4 partial
projections per batch and adds the bias (the unshard step).

Schedule: the ACT engine (softplus = Exp then Ln over every causal score
element, ~143us at 1 elem/lane/cycle) is the roofline for this problem, so the
kernel is built around keeping ACT busy from ~13us on:
  - i-chunk-major pipeline: attention for chunk ic starts as soon as its q/k
    slabs exist; QKV for later chunks, the V tiles, and the output projection
    are emitted as "filler" PE work interleaved between attention blocks,
    paced adaptively and deferred toward the late (ACT-rich) chunks.
  - score/out/norm matmul pairs use disjoint PE row/col tiles (K=64 row-split,
    M=64 / M=1 col-split) so each pair runs concurrently (~1 matmul time).
  - lag-1 software pipeline: unit k+1's scores+softplus are emitted before
    unit k's out/norm matmuls so the PE stream never parks on Ln(k) with
    ACT's next input missing.
  - softplus Exp reads score pairs [128, 2N] straight from 2 PSUM banks; for
    full blocks two j-blocks share one Ln over [128, 4N] (fp16 intermediate)
    to amortize ACT instruction overhead.
  - norm^2 rows for the two head-pairs land in one PSUM bank at partitions
    {0,64} (pair 0) and {32,96} (pair 1); one Ln+Exp epilogue per i-chunk
    computes every rsqrt; out.T is staged unnormalized to SBUF so the po
    PSUM bank recycles immediately.
  - host pre-lays inputs out so every input DMA moves >=2KB contiguous per
    partition line (DMA granularity sweet spot).
PSUM budget: scores [128,2048] (4 banks) + po (1) + pn (1) + aux ring
(qkv/v/proj/bcast, 2) = 8 banks exactly.
"""

import collections
import numpy as np
import ml_dtypes

import concourse.bacc as bacc
import concourse.tile as tile
import concourse.mybir as mybir
from concourse.bass_utils import run_bass_kernel_spmd

# The act-table-set chooser assigns each activation the FIRST set containing
# its function; with the default ordering Exp -> exp_and_others and
# Ln -> natural_log, so alternating Exp/Ln thrashes ACT_TABLE_LOAD (~1.3us
# each, >100 loads). Reorder so the combined Exp+Ln set is preferred.
_orig_get_tables = bacc.get_activation_tables


def _tables_ln_exp_first(arch):
    t = _orig_get_tables(arch)
    key = "natural_log_exp_and_others"
    if key not in t:
        return t
    # Keep dict ORDER (set ids are positional); drop Exp/Ln from every other
    # set so the combined set is the unique candidate for both.
    exp = mybir.ActivationFunctionType.Exp
    ln = mybir.ActivationFunctionType.Ln
    out = {}
    for k, fns in t.items():
        out[k] = fns if k == key else (set(fns) - {exp, ln})
    return out


bacc.get_activation_tables = _tables_ln_exp_first

dt = mybir.dt
F32, F16, BF16 = dt.float32, dt.float16, dt.bfloat16
AF = mybir.ActivationFunctionType

B, T, C, H, HS = 2, 2048, 1024, 16, 64
NCORES = 8
SCALE = 1.0 / np.sqrt(HS)

_CACHE = {}


def _build():
    nc = bacc.Bacc(None, target_bir_lowering=False)

    # host-prearranged layouts: partition dim first, >=2KB contiguous lines
    xT = nc.dram_tensor("xT", [128, 4, 4, 1024], BF16, kind="ExternalInput")
    wqk = nc.dram_tensor("wqk", [128, 4, 1024], BF16, kind="ExternalInput")
    wv = nc.dram_tensor("wv", [128, 2, 1024], BF16, kind="ExternalInput")
    wp = nc.dram_tensor("wp", [128, 2, 1024], BF16, kind="ExternalInput")
    mtri = nc.dram_tensor("mtri", [128, 128], BF16, kind="ExternalInput")
    out = nc.dram_tensor("out", [T, C], F32, kind="ExternalOutput")

    with tile.TileContext(nc) as tc:
        with (
            tc.tile_pool(name="cst", bufs=1) as cst,
            tc.tile_pool(name="data", bufs=1) as data,
            tc.tile_pool(name="ps", bufs=1, space="PSUM") as ps_pool,
            tc.tile_pool(name="pso", bufs=1, space="PSUM") as pso_pool,
            tc.tile_pool(name="psn", bufs=1, space="PSUM") as psn_pool,
            tc.tile_pool(name="aux", bufs=2, space="PSUM") as aux_pool,
            tc.tile_pool(name="we", bufs=3) as we,
            tc.tile_pool(name="epi", bufs=2) as epi,
            tc.tile_pool(name="outp", bufs=6) as outp,
        ):
            # ---- act-table preload: a tiny Exp at t~0 pulls the single
            # ACT_TABLE_LOAD (natural_log_exp_and_others) off the critical path
            warm = cst.tile([1, 2], F32)
            nc.vector.memset(warm, 0.0)
            warm2 = cst.tile([1, 2], F32)
            nc.scalar.activation(warm2, warm, AF.Exp)

            # ---- constants / weights (DMA issue order == need order) ----
            mtri_sb = cst.tile([128, 128], BF16)
            nc.sync.dma_start(mtri_sb, mtri[:])
            ones_n = cst.tile([128, 1], BF16)
            nc.vector.memset(ones_n, 1.0)
            ones_b = cst.tile([128, 64], BF16)
            nc.vector.memset(ones_b, 1.0)

            # wqk_sb[:, cp, c2*512 + jb*128 + j] = W_qkv row (2cp+c2)*128+pi
            wqk_sb = cst.tile([128, 4, 1024], BF16)
            # xT_sb[:, tq, cp, c2*512 + tj] = x.T[(2cp+c2)*128+pi, tq*512+tj]
            xT_sb = data.tile([128, 4, 4, 1024], BF16)
            wv_sb = cst.tile([128, 2, 1024], BF16)
            wp_sb = cst.tile([128, 2, 1024], BF16)
            # first-needed tensors in halves (partial compute start), the
            # rest as single >=0.5MB transfers (descriptor-amortized)
            for h in range(2):
                nc.sync.dma_start(wqk_sb[:, 2 * h:2 * h + 2, :],
                                  wqk[:, 2 * h:2 * h + 2, :])
                nc.sync.dma_start(xT_sb[:, 0, 2 * h:2 * h + 2, :],
                                  xT[:, 0, 2 * h:2 * h + 2, :])
            nc.sync.dma_start(wv_sb, wv[:])
            for tq in range(1, 4):
                nc.sync.dma_start(xT_sb[:, tq, :, :], xT[:, tq, :, :])
            nc.sync.dma_start(wp_sb, wp[:])

            def x_mv(cb, tcc):     # moving [128, 512] for q/k slab
                return xT_sb[:, tcc, cb // 2,
                             (cb % 2) * 512:(cb % 2) * 512 + 512]

            def x_st(cb, tb):      # stationary [128, 128] for v tile
                off = (cb % 2) * 512 + (tb % 4) * 128
                return xT_sb[:, tb // 4, cb // 2, off:off + 128]

            # qkT: block 0,1 = q head-pairs; block 2,3 = k head-pairs.
            # Partition rows (h%2)*64..+64 inside each block = one head.
            qkT = data.tile([128, 4, 2048], BF16)
            v_sb = data.tile([128, 16, 256], BF16)
            yTu = data.tile([128, 2, 2048], BF16)   # unnormalized out.T
            yT = data.tile([128, 2, 2048], BF16)    # normalized out.T

            # scores psum: one 4-bank tile, manually split in two
            # double-buffered halves (range-level deps track the halves).
            ps_all = ps_pool.tile([128, 2048], F32)

            # ---------- PE filler units (QKV slabs / V tiles / proj) -------
            def qk_slab(jb, tcc, t0=0, t1=512):
                pq = aux_pool.tile([128, 512], F32, tag="aux")
                for cb in range(8):
                    nc.tensor.matmul(
                        pq[:, t0:t1],
                        wqk_sb[:, cb // 2,
                               (cb % 2) * 512 + jb * 128:
                               (cb % 2) * 512 + jb * 128 + 128],
                        x_mv(cb, tcc)[:, t0:t1],
                        start=(cb == 0), stop=(cb == 7),
                    )
                nc.vector.tensor_copy(
                    qkT[:, jb, tcc * 512 + t0:tcc * 512 + t1],
                    pq[:, t0:t1])

            def v_tile(tb):
                pv = aux_pool.tile([128, 512], F32, tag="aux")
                for cb in range(8):
                    nc.tensor.matmul(
                        pv[:, 0:256],
                        x_st(cb, tb),
                        wv_sb[:, cb // 4,
                              (cb % 4) * 256:(cb % 4) * 256 + 256],
                        start=(cb == 0), stop=(cb == 7),
                    )
                nc.vector.tensor_copy(v_sb[:, tb, :], pv[:, 0:256])

            def proj_half(tcc, nk, pp=None, scalar_copy=False):
                if pp is None:
                    pp = aux_pool.tile([128, 512], F32, tag="aux")
                for kb in range(2):
                    nc.tensor.matmul(
                        pp,
                        yT[:, kb, tcc * 128:(tcc + 1) * 128],
                        wp_sb[:, kb, nk * 512:(nk + 1) * 512],
                        start=(kb == 0), stop=(kb == 1),
                    )
                os_ = outp.tile([128, 512], F32, tag="os")
                if scalar_copy:
                    nc.scalar.copy(os_, pp)
                else:
                    nc.vector.tensor_copy(os_, pp)
                nc.sync.dma_start(
                    out[tcc * 128:(tcc + 1) * 128, nk * 512:(nk + 1) * 512],
                    os_)

            fillers = collections.deque()   # (key, fn)
            emitted = set()

            def pop_fillers(n):
                for _ in range(n):
                    if not fillers:
                        return
                    key, fn = fillers.popleft()
                    emitted.add(key)
                    fn()

            def ensure(*keys):
                """Emit queued fillers (in order) until every key has run.
                Correctness barrier: consumers call this right before the
                first instruction that reads a filler's output."""
                while any(k not in emitted for k in keys) and fillers:
                    pop_fillers(1)

            # ---------------- attention building blocks -------------------
            def scores_block(hp, ic, jb, half):
                """Score pair for j-block jb against i-chunk ic; returns
                (ps half AP, N)."""
                qblk, kblk = hp, 2 + hp
                m = jb - 4 * ic
                N = 512 if m < 0 else 512 - 128 * m
                ioff = ic * 512 + (512 - N)
                ps_ = ps_all[:, half * 1024:half * 1024 + 1024]
                nc.tensor.matmul(
                    ps_[:, 0:N],
                    qkT[0:64, kblk, jb * 128:(jb + 1) * 128],
                    qkT[0:64, qblk, ioff:ioff + N],
                    start=True, stop=True,
                )
                nc.tensor.matmul(
                    ps_[:, 512:512 + N],
                    qkT[64:128, kblk, jb * 128:(jb + 1) * 128],
                    qkT[64:128, qblk, ioff:ioff + N],
                    start=True, stop=True,
                )
                return ps_, N

            def softplus_single(ps_, N, diag):
                """softplus = Ln(Exp(s/8) + 1), fp16 intermediate; one block.
                A-half at w[:, 0:N], B-half at w[:, N:2N]."""
                e = we.tile([128, 1024], F16, tag="e1")
                w = we.tile([128, 1024], BF16, tag="w1")
                w2 = we.tile([128, 1024], BF16, tag="ww1")
                if N == 512:
                    nc.scalar.activation(e, ps_, AF.Exp, scale=SCALE)
                else:
                    ps3 = ps_.rearrange("p (b n) -> p b n", b=2)[:, :, 0:N]
                    e3 = e[:, 0:2 * N].rearrange("p (b n) -> p b n", b=2)
                    nc.scalar.activation(e3, ps3, AF.Exp, scale=SCALE)
                nc.scalar.activation(w[:, 0:2 * N], e[:, 0:2 * N],
                                     AF.Ln, bias=1.0)
                if diag:
                    nc.vector.tensor_mul(w[:, 0:128], w[:, 0:128], mtri_sb)
                    nc.vector.tensor_mul(w[:, N:N + 128], w[:, N:N + 128],
                                         mtri_sb)
                nc.vector.tensor_mul(w2[:, 0:2 * N], w[:, 0:2 * N],
                                     w[:, 0:2 * N])
                return w, w2

            def out_norm_block(hp, ic, njb, jb, w, w2, woff, po, pn):
                """out.T and norm^2 accumulation for one block; w/w2 tile
                slices start at free offset woff, layout [A(N) | B(N)]."""
                m = jb - 4 * ic
                N = 512 if m < 0 else 512 - 128 * m
                start, stop = jb == 0, jb == njb - 1
                hA, hB = 2 * hp, 2 * hp + 1
                nc.tensor.matmul(
                    po[0:64, 512 - N:512],
                    v_sb[:, jb, hA * 64:hA * 64 + 64],
                    w[:, woff:woff + N],
                    start=start, stop=stop, tile_position=(0, 0),
                )
                nc.tensor.matmul(
                    po[64:128, 512 - N:512],
                    v_sb[:, jb, hB * 64:hB * 64 + 64],
                    w[:, woff + N:woff + 2 * N],
                    start=start, stop=stop, tile_position=(0, 64),
                )
                rA, rB = 32 * hp, 64 + 32 * hp
                nc.tensor.matmul(
                    pn[rA:rA + 1, 512 - N:512], ones_n,
                    w2[:, woff:woff + N],
                    start=start, stop=stop, tile_position=(0, rA),
                )
                nc.tensor.matmul(
                    pn[rB:rB + 1, 512 - N:512], ones_n,
                    w2[:, woff + N:woff + 2 * N],
                    start=start, stop=stop, tile_position=(0, rB),
                )

            # ------------------------- pipeline ---------------------------
            # prologue: only what the very first score block needs — full q
            # slab for pair 0 plus the first 128 k columns
            qk_slab(0, 0)           # q pair 0
            qk_slab(2, 0, 0, 128)   # k pair 0, j-block 0 only
            emitted.update({"s0_0", "s2_0"})
            fillers.append(("k0rest", lambda: qk_slab(2, 0, 128, 512)))
            for tb_ in range(4):
                fillers.append(("v%d" % tb_, lambda tb_=tb_: v_tile(tb_)))
            fillers.append(("s1_0", lambda: qk_slab(1, 0)))   # q pair 1
            fillers.append(("s3_0", lambda: qk_slab(3, 0)))   # k pair 1

            flip = 0
            pending_epi = [None]
            pending = None

            # lag-1 software pipeline (crosses chunk boundaries): emit unit
            # k+1's scores+softplus BEFORE unit k's out/norm matmuls, so the
            # PE stream never parks on Ln(k) with ACT's next input missing.
            def emit_out_norm(pend):
                hp_, jbs_, w_, w2_, po_, ic_, njb_, pn_ = pend
                ensure(*("v%d" % jb_ for jb_ in jbs_))
                for i_, jb_ in enumerate(jbs_):
                    out_norm_block(hp_, ic_, njb_, jb_, w_, w2_, 1024 * i_,
                                   po_, pn_)
                if jbs_[-1] == njb_ - 1:
                    # stage unnormalized out.T; frees po for the next pair
                    nc.vector.tensor_copy(
                        yTu[:, hp_, ic_ * 512:(ic_ + 1) * 512], po_)

            for ic in range(4):
                # queue this chunk's filler PE work; projection is deferred
                # toward the late, ACT-rich chunks.
                if ic > 0:
                    # this chunk's v tiles and pair-1 q/k slabs (due mid-chunk)
                    for tb_ in range(4 * ic, 4 * ic + 4):
                        fillers.append(
                            ("v%d" % tb_, lambda tb_=tb_: v_tile(tb_)))
                    fillers.append(
                        ("s1_%d" % ic, lambda ic=ic: qk_slab(1, ic)))
                    fillers.append(
                        ("s3_%d" % ic, lambda ic=ic: qk_slab(3, ic)))
                if ic < 3:
                    # next chunk's pair-0 q/k slabs (due at its start)
                    fillers.append(
                        ("s0_%d" % (ic + 1),
                         lambda tn=ic + 1: qk_slab(0, tn)))
                    fillers.append(
                        ("s2_%d" % (ic + 1),
                         lambda tn=ic + 1: qk_slab(2, tn)))
                for pic in ((0,) if ic == 2 else (1, 2) if ic == 3 else ()):
                    for tcc_ in range(4 * pic, 4 * pic + 4):
                        for nk_ in range(2):
                            fillers.append(
                                ("p%d_%d" % (tcc_, nk_),
                                 lambda tcc_=tcc_, nk_=nk_:
                                 proj_half(tcc_, nk_)))

                njb = 4 * ic + 4
                units_left = 2 * (2 * ic + 4)
                pn = psn_pool.tile([128, 512], F32, tag="pn")

                first_unit = True
                for hp in range(2):
                    po = pso_pool.tile([128, 512], F32, tag="po")
                    jb = 0
                    while jb < njb:
                        jb_max = jb + 1 if jb + 1 < 4 * ic else jb
                        ensure("s%d_%d" % (hp, ic),
                               "s%d_%d" % (2 + hp, jb_max // 4))
                        if hp == 0 and jb > 0:
                            ensure("k0rest")
                        if jb + 1 < 4 * ic:  # merge two full blocks
                            psA, _ = scores_block(hp, ic, jb, 0)
                            psB, _ = scores_block(hp, ic, jb + 1, 1)
                            # per-half Exp (finer WAR release of the score
                            # banks), one Ln over both halves
                            e = we.tile([128, 2048], F16, tag="e2")
                            w = we.tile([128, 2048], BF16, tag="w2")
                            w2 = we.tile([128, 2048], BF16, tag="ww2")
                            nc.scalar.activation(e, ps_all, AF.Exp,
                                                 scale=SCALE)
                            nc.scalar.activation(w, e, AF.Ln, bias=1.0)
                            nc.vector.tensor_mul(w2, w, w)
                            cur = (hp, (jb, jb + 1), w, w2, po, ic, njb, pn)
                            jb += 2
                        else:
                            m = jb - 4 * ic
                            ps_, N = scores_block(hp, ic, jb, flip)
                            flip ^= 1
                            w, w2 = softplus_single(ps_, N, m >= 0)
                            cur = (hp, (jb,), w, w2, po, ic, njb, pn)
                            jb += 1
                        if first_unit and pending_epi[0] is not None:
                            # previous chunk's epilogue, now that ACT has
                            # this chunk's first Exp queued ahead of it
                            pending_epi[0]()
                            pending_epi[0] = None
                        first_unit = False
                        # adaptive pacing: leave no backlog for chunk end
                        quota = max(1, -(-len(fillers) // max(units_left, 1)))
                        pop_fillers(min(quota, 2))
                        if pending is not None:
                            emit_out_norm(pending)
                            pop_fillers(1 if quota > 2 else 0)
                        pending = cur
                        units_left -= 1
                emit_out_norm(pending)
                pending = None

                # anything not yet emitted must land before the next chunk
                pop_fillers(len(fillers))

                # ---- chunk epilogue: rsqrt(norm2) for all 4 heads ----
                # rsqrt = Exp(-0.5 * Ln(x)): stays in the Exp/Ln set.
                # Rows {0,64} = pair 0, {32,96} = pair 1; unread rows compute
                # garbage harmlessly (ACT cost is free-dim bound). Deferred
                # until the next chunk's first Exp is queued so the pb
                # matmuls (which park on ACT's rsqrt) never delay it.
                def chunk_epilogue(ic=ic, pn=pn):
                    nrm = epi.tile([128, 512], F32, tag="nrm")
                    nc.scalar.activation(nrm, pn, AF.Ln)
                    rs = epi.tile([128, 512], BF16, tag="rs")
                    nc.scalar.activation(rs, nrm, AF.Exp, scale=-0.5)
                    for hp in range(2):
                        rA, rB = 32 * hp, 64 + 32 * hp
                        pb = aux_pool.tile([128, 512], F32, tag="aux")
                        nc.tensor.matmul(pb[0:64, :], ones_b[rA:rA + 1, :],
                                         rs[rA:rA + 1, :],
                                         start=True, stop=True,
                                         tile_position=(rA, 0))
                        nc.tensor.matmul(pb[64:128, :], ones_b[rB:rB + 1, :],
                                         rs[rB:rB + 1, :],
                                         start=True, stop=True,
                                         tile_position=(rB, 64))
                        rb = epi.tile([128, 512], F32, tag="rb")
                        nc.vector.tensor_copy(rb, pb)
                        nc.vector.tensor_mul(
                            yT[:, hp, ic * 512:(ic + 1) * 512],
                            yTu[:, hp, ic * 512:(ic + 1) * 512], rb)

                if ic < 3:
                    pending_epi[0] = chunk_epilogue
                else:
                    chunk_epilogue()

                # anything not yet emitted must land before the next chunk
                pop_fillers(len(fillers))

            # last chunk's projection: rotate the accumulator through the
            # aux ring plus the now-idle po/pn banks (4-deep pipeline with
            # proper per-tile release, unlike slicing one big psum tile).
            tail_pp = [
                lambda: aux_pool.tile([128, 512], F32, tag="aux", name="tpa"),
                lambda: aux_pool.tile([128, 512], F32, tag="aux", name="tpb"),
                lambda: pso_pool.tile([128, 512], F32, tag="po", name="tpo"),
                lambda: psn_pool.tile([128, 512], F32, tag="pn", name="tpn"),
            ]
            k = 0
            for tcc_ in range(12, 16):
                for nk_ in range(2):
                    proj_half(tcc_, nk_, pp=tail_pp[k % 4]())
                    k += 1

    nc.compile()
    return nc


def _prep_inputs(x, W_qkv, W_proj):
    """Host-side shard + layout prep. Returns per-core input maps.

    Layouts are DMA-friendly: partition index first, then chunk indices so
    every dma_start moves a contiguous >=2KB line per partition.
    """
    bf = ml_dtypes.bfloat16
    mtri = np.triu(np.ones((128, 128), dtype=np.float32)).astype(bf)
    in_maps = []
    for core in range(NCORES):
        b, g = core // 4, core % 4
        heads = range(4 * g, 4 * g + 4)
        # W_qkv rows: q = h*64.., k = C + h*64.., v = 2C + h*64..
        q_rows = np.concatenate([np.arange(h * HS, (h + 1) * HS) for h in heads])
        wqk = np.concatenate(
            [W_qkv[q_rows, :].T, W_qkv[C + q_rows, :].T], axis=1)  # [C, 512]
        wv = W_qkv[2 * C + q_rows, :].T                            # [C, 256]
        wp = W_proj[:, q_rows].T                                   # [256, C]

        xT_ = np.ascontiguousarray(x[b].T).astype(bf)              # [C, T]
        # [128, tq, cp, c2*512+tj]
        xh = xT_.reshape(4, 2, 128, 4, 512).transpose(2, 3, 0, 1, 4)
        xh = np.ascontiguousarray(xh).reshape(128, 4, 4, 1024)
        # [128, cp, c2*512+j]
        wqkh = wqk.astype(bf).reshape(4, 2, 128, 512).transpose(2, 0, 1, 3)
        wqkh = np.ascontiguousarray(wqkh).reshape(128, 4, 1024)
        # [128, cq, c4*256+j]
        wvh = wv.astype(bf).reshape(2, 4, 128, 256).transpose(2, 0, 1, 3)
        wvh = np.ascontiguousarray(wvh).reshape(128, 2, 1024)
        # [128, po, e]
        wph = np.ascontiguousarray(
            wp.astype(bf).reshape(2, 128, 1024).transpose(1, 0, 2))

        in_maps.append({
            "xT": xh, "wqk": wqkh, "wv": wvh, "wp": wph, "mtri": mtri,
        })
    return in_maps


def _run(in_maps, trace=False, trace_cores=None):
    if "nc" not in _CACHE:
        _CACHE["nc"] = _build()
    return run_bass_kernel_spmd(
        _CACHE["nc"], in_maps, core_ids=list(range(NCORES)),
        trace=trace, trace_cores=trace_cores,
    )


def kernel(x, W_qkv, W_proj, b_proj):
    x = np.asarray(x, dtype=np.float32)
    W_qkv = np.asarray(W_qkv, dtype=np.float32)
    W_proj = np.asarray(W_proj, dtype=np.float32)
    b_proj = np.asarray(b_proj, dtype=np.float32)

    res = _run(_prep_inputs(x, W_qkv, W_proj)).results
    out = np.zeros((B, T, C), dtype=np.float64)
    for core in range(NCORES):
        out[core // 4] += np.asarray(res[core]["out"], dtype=np.float64)
    out += b_proj.astype(np.float64)
    return out.astype(np.float32)
